# revision 6
# baseline (speedup 1.0000x reference)
"""Trainium2 Bass kernel for nn_MetaLEAPPredictor (GNN edge scoring).

reference:
    w0   = sf @ psi_w.T + psi_b                      # [E, 2C]
    coef = w0 + delta_w[li] + u[li]
    s    = sum(x[row] * coef[:, :C], -1) + sum(x[col] * coef[:, C:], -1)
    y    = gamma_h[li][None, :] * leaky_relu(s, 0.01)[:, None]

Algebraic restructure: with b0 = psi_b + delta_w[li] + u[li],
    s_e = <[sf_e, 1], T[row_e, 0:5]> + <[sf_e, 1], T[col_e, 8:13]>
where T = x @ Wext, Wext[c, 0:4] = psi_w[c, :], Wext[c, 4] = b0[c],
Wext[c, 8:12] = psi_w[64+c, :], Wext[c, 12] = b0[64+c]   (c in [0, 64)).

Device plan (8 cores, edges sharded):
  Phase A (each core, full node set): cast x to bf16 (SWDGE cast-DMA),
  DMA-transpose (xbar) packed pairs, PE matmuls vs Wext (bf16) -> PSUM,
  copy to SBUF staging, write table [100096, 64] f32 rows to HBM
  (row index interleaved: trow(n) = ((n%2)*64 + (n%128)//2)*782 + n//128).
  Phase B: per-edge gather of 64B table rows via InstDMAGatherAnt
  (1024 int16 idx per instruction; 16 buckets = row-chunk x col-chunk of
  32768 rows so indices fit int16), then DVE mul/reduce/leaky/broadcast.
Host only shards, buckets, pads, and unpermutes.
"""
import sys
if '/opt/trn_rl_repo' not in sys.path:
    sys.path.insert(0, '/opt/trn_rl_repo')

import numpy as np
import ml_dtypes

import concourse.bacc as bacc
import concourse.bass as bass
import concourse.mybir as mybir
from concourse import tile
from concourse import ap_utils
from concourse.bass import exact_div, round_up_to_multiple
from concourse.library_config import mlp
from concourse.bass_utils import run_bass_kernel_spmd

N = 100000
C = 64
E = 1600000
H = 8
NEG = 0.01
NCORES = 8
NP = 100096            # N padded to 128*782
TILES = NP // 128      # 782
CHUNK = 32768          # gather chunk (int16 range)
NCHUNKS = 4            # ceil(100096 / 32768)
GT = 1024              # edges per gather instruction
GS = 8                 # gather tiles per DVE group


def _dma_gather_raw(gp, out_ap, in_ap, idxs_ap, num_idxs, elem_size, elem_step):
    """bass.dma_gather minus the (transpose-only) elem%256 assert."""
    assert idxs_ap.dtype == mybir.dt.int16
    assert in_ap.dtype == out_ap.dtype
    assert in_ap.space == bass.MemorySpace.DRAM
    assert ap_utils.ap_is_contiguous(out_ap.ap[1:])
    assert ap_utils.ap_is_contiguous(idxs_ap.ap[1:])
    assert in_ap.ap[-1][1] == out_ap.ap[-1][1] == elem_size
    assert out_ap.ap[0][1] * out_ap.ap[1][1] == round_up_to_multiple(num_idxs, 128)
    assert in_ap.ap[0][0] == elem_step
    stride_bytes_256 = exact_div(elem_step * mybir.dt.size(in_ap.dtype), 256)
    _in_ap = gp.lower_ap_dma(in_ap, for_custom_bir_dma=True)
    return gp.add_instruction(
        mybir.InstDMAGatherAnt(
            name=gp.bass.get_next_instruction_name(),
            ins=[*_in_ap, gp.lower_ap(idxs_ap),
                 gp.lower_val_access(gp.to_reg(num_idxs))],
            outs=[gp.lower_ap(out_ap)],
            transpose=False, num_idxs=num_idxs, elem_size=elem_size,
            stride_bytes_256=stride_bytes_256, gen_mode=0, single_packet=True,
            queue_num=0, sbuf_tokens_per_rank=0, sbuf_free_dim_per_rank=0,
            sbuf_free_dim_pad_per_rank=0, sbuf_byte_offset=0,
        ))


def build_program(groups, nrep=1):
    """Build the SPMD Bass program.

    groups: flat list of (bucket, gs) with gs = gather tiles in the group.
    """
    ngroups = len(groups)
    nc = bacc.Bacc("TRN2", target_bir_lowering=False, debug=False,
                   num_devices=NCORES)

    x32 = nc.dram_tensor("x32", [N, C], mybir.dt.float32, kind="ExternalInput")
    w64 = nc.dram_tensor("w64", [C, 64], mybir.dt.bfloat16, kind="ExternalInput")
    gamma = nc.dram_tensor("gamma", [128, H], mybir.dt.float32, kind="ExternalInput")
    idxr = nc.dram_tensor("idxr", [ngroups, 128, GS * (GT // 16)],
                          mybir.dt.int16, kind="ExternalInput")
    idxc = nc.dram_tensor("idxc", [ngroups, 128, GS * (GT // 16)],
                          mybir.dt.int16, kind="ExternalInput")
    sfd = nc.dram_tensor("sfd", [ngroups, 128, GS * (GT // 128) * 4],
                         mybir.dt.float32, kind="ExternalInput")
    ydev = nc.dram_tensor("ydev", [ngroups, 128, GS * (GT // 128) * H],
                          mybir.dt.float32, kind="ExternalOutput")
    xbf = nc.dram_tensor("xbf", [NP * C], mybir.dt.bfloat16)
    table = nc.dram_tensor("table", [NP, 64], mybir.dt.float32)

    xbf2d = xbf[:].rearrange("(r c) -> r c", c=C)            # [NP, 64]
    xbf_pack = xbf[:].rearrange("(r c) -> r c", c=2 * C)     # [NP//2, 128]
    table_v = table[:].rearrange("(p t) c -> p t c", t=TILES)  # [128, 782, 64]

    SL = GT // 128      # 8 slots per partition per gather tile
    IW = GT // 16       # 64 idx words per partition per tile

    with tile.TileContext(nc) as tc:
        with tc.tile_critical():
            nc.gpsimd.load_library(mlp)
        with tc.tile_pool(name="cast", bufs=2) as cast_pool, \
             tc.tile_pool(name="xt", bufs=3) as xt_pool, \
             tc.tile_pool(name="stage", bufs=2) as stage_pool, \
             tc.tile_pool(name="wp", bufs=1) as wp, \
             tc.tile_pool(name="gath", bufs=3) as gath_pool, \
             tc.tile_pool(name="io", bufs=3) as io_pool, \
             tc.tile_pool(name="cmp", bufs=3) as cmp_pool, \
             tc.tile_pool(name="psum", bufs=4, space="PSUM") as psum_pool:

            wt = wp.tile([128, 64], mybir.dt.bfloat16)
            nc.sync.dma_start(out=wt[0:C, :], in_=w64[:])
            nc.sync.dma_start(out=wt[C:2 * C, :], in_=w64[:])
            gm = wp.tile([128, H], mybir.dt.float32)
            nc.sync.dma_start(out=gm[:], in_=gamma[:])
            # zero the xbf pad rows (nodes N..NP) so matmuls stay finite
            zpad = wp.tile([128, (NP - N) * C // 128], mybir.dt.bfloat16)
            nc.vector.memset(zpad[:], 0.0)
            nc.sync.dma_start(
                out=xbf[N * C:].rearrange("(p c) -> p c", p=128), in_=zpad[:])

            for rep in range(nrep):
                # ---- Phase A: build table ----
                ncast = 8
                rows_per = (N + ncast - 1) // ncast
                for i in range(ncast):
                    r0, r1 = i * rows_per, min((i + 1) * rows_per, N)
                    nc.gpsimd.dma_start(out=xbf2d[r0:r1, :], in_=x32[r0:r1, :])

                PT = 8  # node-tiles per psum bank / transpose group
                ngA = (TILES + PT - 1) // PT   # 98 groups
                for g in range(ngA):
                    t0 = g * PT
                    nt = min(PT, TILES - t0)          # node tiles this group
                    prows = nt * 64                   # packed rows
                    xt = xt_pool.tile([128, PT * 64], mybir.dt.bfloat16, tag="xt")
                    nc.sync.dma_start_transpose(
                        out=xt[:, :prows], in_=xbf_pack[t0 * 64: t0 * 64 + prows, :])
                    ps = psum_pool.tile([128, PT * 64], mybir.dt.float32, tag="ps")
                    for j in range(nt):
                        o = j * 64
                        nc.tensor.matmul(
                            out=ps[0:64, o:o + 64], lhsT=xt[0:C, o:o + 64],
                            rhs=wt[0:C, :], start=True, stop=True)
                        nc.tensor.matmul(
                            out=ps[64:128, o:o + 64], lhsT=xt[C:2 * C, o:o + 64],
                            rhs=wt[C:2 * C, :], start=True, stop=True)
                    st = stage_pool.tile([128, PT * 64], mybir.dt.float32, tag="st")
                    nc.vector.tensor_copy(out=st[:, :nt * 64], in_=ps[:, :nt * 64])
                    nc.sync.dma_start(
                        out=table_v[:, t0:t0 + nt, :],
                        in_=st[:, :nt * 64].rearrange("p (t c) -> p t c", c=64))

                # ---- Phase B: gather + per-edge compute ----
                for g, (b, gs) in enumerate(groups):
                        rc, cc = divmod(b, NCHUNKS)
                        rlo = rc * CHUNK
                        clo = cc * CHUNK
                        rhi = min(NP, rlo + CHUNK) - rlo
                        chi = min(NP, clo + CHUNK) - clo
                        src_r = table[rlo:rlo + rhi, 0:16]
                        src_c = table[clo:clo + chi, 0:16]
                        ir = io_pool.tile([128, GS * IW], mybir.dt.int16, tag="ir")
                        ic = io_pool.tile([128, GS * IW], mybir.dt.int16, tag="ic")
                        nc.sync.dma_start(out=ir[:, :gs * IW],
                                          in_=idxr[g, :, :gs * IW])
                        nc.sync.dma_start(out=ic[:, :gs * IW],
                                          in_=idxc[g, :, :gs * IW])
                        sft = io_pool.tile([128, GS * SL * 4], mybir.dt.float32,
                                           tag="sf")
                        nc.sync.dma_start(out=sft[:, :gs * SL * 4],
                                          in_=sfd[g, :, :gs * SL * 4])
                        gr = gath_pool.tile([128, GS, SL, 16], mybir.dt.float32,
                                            tag="gr")
                        gc = gath_pool.tile([128, GS, SL, 16], mybir.dt.float32,
                                            tag="gc")
                        for t in range(gs):
                            _dma_gather_raw(
                                nc.gpsimd, gr[:, t], src_r,
                                ir[:, t * IW:(t + 1) * IW], GT, 16, 64)
                            _dma_gather_raw(
                                nc.gpsimd, gc[:, t], src_c,
                                ic[:, t * IW:(t + 1) * IW], GT, 16, 64)
                        # compute on [128, gs, SL, *]
                        sf4 = sft[:, :gs * SL * 4].rearrange(
                            "p (t s k) -> p (t s) k", k=4, s=SL)
                        grv = gr[:, :gs].rearrange("p t s e -> p (t s) e")
                        gcv = gc[:, :gs].rearrange("p t s e -> p (t s) e")
                        pr = cmp_pool.tile([128, GS * SL, 4], mybir.dt.float32,
                                           tag="pr")
                        pc = cmp_pool.tile([128, GS * SL, 4], mybir.dt.float32,
                                           tag="pc")
                        nv = gs * SL
                        nc.vector.tensor_tensor(out=pr[:, :nv], in0=sf4,
                                                in1=grv[:, :, 0:4],
                                                op=mybir.AluOpType.mult)
                        nc.vector.tensor_tensor(out=pc[:, :nv], in0=sf4,
                                                in1=gcv[:, :, 8:12],
                                                op=mybir.AluOpType.mult)
                        s0 = cmp_pool.tile([128, GS * SL], mybir.dt.float32,
                                           tag="s0")
                        s1 = cmp_pool.tile([128, GS * SL], mybir.dt.float32,
                                           tag="s1")
                        nc.vector.tensor_reduce(out=s0[:, :nv], in_=pr[:, :nv],
                                                axis=mybir.AxisListType.X,
                                                op=mybir.AluOpType.add)
                        nc.vector.tensor_reduce(out=s1[:, :nv], in_=pc[:, :nv],
                                                axis=mybir.AxisListType.X,
                                                op=mybir.AluOpType.add)
                        nc.vector.tensor_tensor(out=s0[:, :nv], in0=s0[:, :nv],
                                                in1=s1[:, :nv],
                                                op=mybir.AluOpType.add)
                        nc.vector.tensor_tensor(out=s0[:, :nv], in0=s0[:, :nv],
                                                in1=grv[:, :, 4:5].squeeze(2),
                                                op=mybir.AluOpType.add)
                        nc.vector.tensor_tensor(out=s0[:, :nv], in0=s0[:, :nv],
                                                in1=gcv[:, :, 12:13].squeeze(2),
                                                op=mybir.AluOpType.add)
                        nc.scalar.mul(s1[:, :nv], s0[:, :nv], NEG)
                        nc.vector.tensor_tensor(out=s0[:, :nv], in0=s0[:, :nv],
                                                in1=s1[:, :nv],
                                                op=mybir.AluOpType.max)
                        yt = cmp_pool.tile([128, GS * SL, H], mybir.dt.float32,
                                           tag="yt")
                        nc.vector.tensor_tensor(
                            out=yt[:, :nv],
                            in0=s0[:, :nv].unsqueeze(2).broadcast_to([128, nv, H]),
                            in1=gm[:].unsqueeze(1).broadcast_to([128, nv, H]),
                            op=mybir.AluOpType.mult)
                        nc.sync.dma_start(
                            out=ydev[g, :, :nv * H],
                            in_=yt[:, :nv].rearrange("p s h -> p (s h)"))
    nc.compile()
    return nc


def _trow(n):
    """table row index for node n (even/odd matmul interleave)."""
    m = n % 128
    return ((m % 2) * 64 + m // 2) * TILES + n // 128


def prep_inputs(x, edge_index, structural_features, layer_idx,
                psi_w, psi_b, delta_w, u, gamma_h):
    li = int(layer_idx)
    b0 = (psi_b + delta_w[li] + u[li]).astype(np.float32)       # [2C]
    w64 = np.zeros((C, 64), dtype=np.float32)
    w64[:, 0:4] = psi_w[:C]
    w64[:, 4] = b0[:C]
    w64[:, 8:12] = psi_w[C:]
    w64[:, 12] = b0[C:]
    w64 = w64.astype(ml_dtypes.bfloat16)
    gamma = np.tile(np.asarray(gamma_h[li], np.float32)[None, :], (128, 1))

    row = np.asarray(edge_index[0], np.int64)
    col = np.asarray(edge_index[1], np.int64)
    sf = np.asarray(structural_features, np.float32)
    x = np.asarray(x, np.float32)

    epc = E // NCORES
    rowt = _trow(row).astype(np.int32)
    colt = _trow(col).astype(np.int32)
    bucket = (rowt // CHUNK) * NCHUNKS + (colt // CHUNK)
    nbuck = NCHUNKS * NCHUNKS
    SL, IW = GT // 128, GT // 16

    cores = []
    cnts = np.zeros((NCORES, nbuck), dtype=np.int64)
    for c in range(NCORES):
        sl = slice(c * epc, (c + 1) * epc)
        order = np.argsort(bucket[sl], kind='stable') + c * epc
        cnts[c] = np.bincount(bucket[sl], minlength=nbuck)
        cores.append(order)
    tpb_b = np.maximum(1, -(-cnts.max(axis=0) // GT))           # [nbuck]
    groups = []
    for b in range(nbuck):
        t = int(tpb_b[b])
        while t > 0:
            gs = min(GS, t)
            groups.append((b, gs))
            t -= gs
    ngroups = len(groups)

    in_maps = []
    eid_all = []
    for c in range(NCORES):
        order = cores[c]
        idxr_d = np.zeros((ngroups, 128, GS * IW), dtype=np.int16)
        idxc_d = np.zeros((ngroups, 128, GS * IW), dtype=np.int16)
        sfd_d = np.zeros((ngroups, 128, GS * SL * 4), dtype=np.float32)
        eids_d = np.full((ngroups, GS * GT), -1, dtype=np.int64)
        boff = np.concatenate([[0], np.cumsum(cnts[c])])
        tile_done = {b: 0 for b in range(nbuck)}
        for g, (b, gs) in enumerate(groups):
            t0 = tile_done[b]
            tile_done[b] = t0 + gs
            lo = boff[b] + t0 * GT
            hi = min(boff[b + 1], lo + gs * GT)
            cnt = max(0, int(hi - lo))
            ids = order[lo:hi]
            npad = gs * GT
            rl = np.zeros(npad, dtype=np.int16)
            cl = np.zeros(npad, dtype=np.int16)
            rl[:cnt] = (rowt[ids] % CHUNK).astype(np.int16)
            cl[:cnt] = (colt[ids] % CHUNK).astype(np.int16)
            eids_d[g, :cnt] = ids
            sfp = np.zeros((npad, 4), dtype=np.float32)
            sfp[:cnt] = sf[ids]
            # idx wrap: [gs, GT] -> per tile [16, IW] replicated to 128
            def wrap(a):
                a = a.reshape(gs, IW, 16).transpose(0, 2, 1)    # [gs, 16, IW]
                a = np.tile(a, (1, 8, 1))                       # [gs, 128, IW]
                return a.transpose(1, 0, 2).reshape(128, gs * IW)
            idxr_d[g, :, :gs * IW] = wrap(rl)
            idxc_d[g, :, :gs * IW] = wrap(cl)
            # sf: edge m = s*128+p of tile t
            sfd_d[g, :, :gs * SL * 4] = (
                sfp.reshape(gs, SL, 128, 4).transpose(2, 0, 1, 3)
                .reshape(128, gs * SL * 4))
        in_maps.append({
            "x32": x, "w64": w64, "gamma": gamma,
            "idxr": idxr_d, "idxc": idxc_d, "sfd": sfd_d,
        })
        eid_all.append(eids_d)
    return in_maps, eid_all, groups


def unshard(results, eid_all, groups):
    SL = GT // 128
    y = np.empty((E, H), dtype=np.float32)
    for c in range(NCORES):
        yd = results[c]["ydev"]          # [ngroups, 128, GS*SL*H]
        eids = eid_all[c]                # [ngroups, GS*GT]
        for g, (b, gs) in enumerate(groups):
            blk = yd[g, :, :gs * SL * H].reshape(128, gs, SL, H)
            blk = blk.transpose(1, 2, 0, 3).reshape(gs * GT, H)
            ids = eids[g, :gs * GT]
            valid = ids >= 0
            y[ids[valid]] = blk[valid]
    return y


_CACHE = {}


def kernel(**inputs):
    in_maps, eid_all, groups = prep_inputs(**inputs)
    key = tuple(groups)
    if key not in _CACHE:
        _CACHE[key] = build_program(groups)
    nc = _CACHE[key]
    res = run_bass_kernel_spmd(nc, in_maps, core_ids=list(range(NCORES)))
    return unshard(res.results, eid_all, groups)
